# revision 10
# baseline (speedup 1.0000x reference)
"""Bi-directional WKV (RWKV-style) kernel for Trainium2, 8-core batch-parallel.

Math per (b, t, c):
    tf_b     = sigmoid(sum(time_emb[b]))
    decay_bc = exp(-exp(w_c)) * (0.5 + 0.5*tf_b)
    ek   = exp(k);  ekv = ek * v
    Fn_t = decay*Fn_{t-1} + ekv_t    (forward inclusive scan over T)
    Fd_t = decay*Fd_{t-1} + ek_t
    Bn_t = decay*Bn_{t+1} + ekv_t    (backward inclusive scan)
    Bd_t = decay*Bd_{t+1} + ek_t
    num  = Fn + Bn + (e^u - 1)*ekv
    den  = Fd + Bd + (e^u - 1)*ek
    out  = sigmoid(r) * num/den * (0.8 + 0.2*tf_b)

Engine split per 128-channel ctile. GpSimd is deliberately UNUSED in the
steady state: concurrent GpSimd elementwise traffic slows Vector scans by
~1.8x (mutual SBUF contention, measured), so offloading to it is a net
loss. Split:
  Vector : ekv mult, the 4 scans, num stt+add, s add, o1 mult.
  Tensor : 128x128 input transposes (k,v,r), output transposes, and the
           den assembly as PSUM-accumulated matmuls
           (diag(c1)@ek + I@Fd + I@Bd).
  Scalar : exp(k), exp(-r), ln(1+e^-r), ln(den) from PSUM,
           expo = exp(-s + lnscale), PSUM->SBUF out copy.
"""

import numpy as np
from contextlib import ExitStack

import concourse.bass as bass
import concourse.bacc as bacc
import concourse.tile as tile
from concourse import mybir
from concourse.bass_utils import run_bass_kernel_spmd
from concourse.masks import make_identity

from concourse.hw_specs import get_activation_tables


def _pin_act_tables():
    tabs = get_activation_tables("gen3")
    keep = {"natural_log_exp_and_others", "sigmoid_and_friends"}
    for name in list(tabs):
        if name not in keep:
            tabs[name] = set()


_pin_act_tables()

B, T, C, TD = 8, 2048, 2048, 512
P = 128
N_CORES = 8
f32 = mybir.dt.float32
ALU = mybir.AluOpType
AF = mybir.ActivationFunctionType


def _body(tc, out, r, k, v, w, u, te, T_, C_, TD_):
    nc = tc.nc
    NT = T_ // P   # number of t-chunks in a slab
    NCT = C_ // P  # number of c-tiles

    with ExitStack() as ctx:
        consts = ctx.enter_context(tc.tile_pool(name="consts", bufs=1))
        slabs = ctx.enter_context(tc.tile_pool(name="slabs", bufs=2))
        oslabs = ctx.enter_context(tc.tile_pool(name="oslabs", bufs=2))
        front = ctx.enter_context(tc.tile_pool(name="front", bufs=2))
        scano = ctx.enter_context(tc.tile_pool(name="scano", bufs=2))
        mid = ctx.enter_context(tc.tile_pool(name="mid", bufs=1))
        stage = ctx.enter_context(tc.tile_pool(name="stage", bufs=2))
        psin = ctx.enter_context(tc.tile_pool(name="psin", bufs=2,
                                              space="PSUM"))
        psdn = ctx.enter_context(tc.tile_pool(name="psdn", bufs=1,
                                              space="PSUM"))
        pso = ctx.enter_context(tc.tile_pool(name="pso", bufs=1,
                                             space="PSUM"))

        ident = consts.tile([P, P], f32)
        make_identity(nc, ident[:])

        # ---- per-batch time factor, on all 128 partitions ----
        te_t = consts.tile([P, TD_], f32)
        te_b = bass.AP(tensor=te.tensor, offset=te.offset,
                       ap=[[0, P]] + list(te.ap))
        nc.sync.dma_start(out=te_t[:], in_=te_b)
        ssum = consts.tile([P, 1], f32)
        nc.vector.tensor_reduce(out=ssum[:], in_=te_t[:],
                                axis=mybir.AxisListType.X, op=ALU.add)
        tf = consts.tile([P, 1], f32)
        nc.scalar.activation(out=tf[:], in_=ssum[:], func=AF.Sigmoid)
        scale_b = consts.tile([P, 1], f32)   # 0.8 + 0.2*tf
        nc.vector.tensor_scalar(out=scale_b[:], in0=tf[:], scalar1=0.2,
                                scalar2=0.8, op0=ALU.mult, op1=ALU.add)
        htf = consts.tile([P, 1], f32)       # 0.5 + 0.5*tf
        nc.vector.tensor_scalar(out=htf[:], in0=tf[:], scalar1=0.5,
                                scalar2=0.5, op0=ALU.mult, op1=ALU.add)

        # ---- per-channel constants, [128, NCT] ----
        wt = consts.tile([P, NCT], f32)
        nc.sync.dma_start(out=wt[:], in_=w.rearrange("(j p) -> p j", p=P))
        ut = consts.tile([P, NCT], f32)
        nc.sync.dma_start(out=ut[:], in_=u.rearrange("(j p) -> p j", p=P))
        ew = consts.tile([P, NCT], f32)
        nc.scalar.activation(out=ew[:], in_=wt[:], func=AF.Exp)        # e^w
        dec0 = consts.tile([P, NCT], f32)
        nc.scalar.activation(out=dec0[:], in_=ew[:], func=AF.Exp,
                             scale=-1.0)                               # e^-e^w
        decay = consts.tile([P, NCT], f32)
        nc.vector.tensor_scalar(out=decay[:], in0=dec0[:],
                                scalar1=htf[:, 0:1], scalar2=None,
                                op0=ALU.mult)
        eu = consts.tile([P, NCT], f32)
        nc.scalar.activation(out=eu[:], in_=ut[:], func=AF.Exp)
        c1 = consts.tile([P, NCT], f32)      # e^u - 1
        nc.vector.tensor_scalar(out=c1[:], in0=eu[:], scalar1=1.0,
                                scalar2=None, op0=ALU.subtract)
        lnscale = consts.tile([P, 1], f32)   # ln(0.8 + 0.2*tf)
        nc.scalar.activation(out=lnscale[:], in_=scale_b[:], func=AF.Ln)

        # DRAM views: (tc tp) (j cc) -> tp tc j cc
        def slab_src(ap, j):
            return ap.rearrange("(tc tp) (j cc) -> tp tc j cc",
                                tp=P, cc=P)[:, :, j, :]

        CH = 512          # psum chunk width (one bank)
        NCH = T_ // CH    # chunks per ctile
        BPC = CH // P     # 128-blocks per chunk

        def load_slabs(j):
            kslab = slabs.tile([P, NT, P], f32, tag="kslab")
            vslab = slabs.tile([P, NT, P], f32, tag="vslab")
            rslab = slabs.tile([P, NT, P], f32, tag="rslab")
            nc.sync.dma_start(out=kslab[:], in_=slab_src(k, j))
            nc.sync.dma_start(out=vslab[:], in_=slab_src(v, j))
            nc.sync.dma_start(out=rslab[:], in_=slab_src(r, j))
            return kslab, vslab, rslab

        def run_front(j, slabs3):
            """Transposes + ek/ekv/l1p + the den scans (fd, bd)."""
            kslab, vslab, rslab = slabs3
            cj = c1[:, j:j + 1]
            djb = decay[:, j:j + 1].broadcast_to((P, T_))
            ek = front.tile([P, T_], f32, tag="ek")
            ekv = front.tile([P, T_], f32, tag="ekv")
            l1p = front.tile([P, T_], f32, tag="l1p")
            diagc = front.tile([P, P], f32, tag="diagc")
            nc.vector.tensor_scalar(out=diagc[:], in0=ident[:], scalar1=cj,
                                    scalar2=None, op0=ALU.mult)
            for q in range(NCH):
                sl = slice(q * CH, (q + 1) * CH)
                pk = psin.tile([P, CH], f32, tag="pk")
                pv = psin.tile([P, CH], f32, tag="pv")
                pr = psin.tile([P, CH], f32, tag="pr")
                for s in range(BPC):
                    tcb = q * BPC + s
                    bs = slice(s * P, (s + 1) * P)
                    nc.tensor.transpose(pk[:, bs], kslab[:, tcb, :], ident[:])
                    nc.tensor.transpose(pv[:, bs], vslab[:, tcb, :], ident[:])
                    nc.tensor.transpose(pr[:, bs], rslab[:, tcb, :], ident[:])
                nc.scalar.activation(out=ek[:, sl], in_=pk[:], func=AF.Exp)
                # stage v to SBUF on Scalar so the Vector mult is all-SBUF
                # (PSUM-read tt measured ~1.4x slower than SBUF-read)
                vS = stage.tile([P, CH], f32, tag="vS")
                nc.scalar.activation(out=vS[:], in_=pv[:], func=AF.Copy)
                nc.vector.tensor_tensor(ekv[:, sl], ek[:, sl], vS[:],
                                        ALU.mult)
                nc.scalar.activation(out=l1p[:, sl], in_=pr[:], func=AF.Exp,
                                     scale=-1.0)
                nc.scalar.activation(out=l1p[:, sl], in_=l1p[:, sl],
                                     func=AF.Ln, bias=1.0)
            fd = scano.tile([P, T_], f32, tag="fd")
            bd = scano.tile([P, T_], f32, tag="bd")
            nc.vector.tensor_tensor_scan(out=fd[:], data0=djb, data1=ek[:],
                                         initial=0.0, op0=ALU.mult,
                                         op1=ALU.add)
            nc.vector.tensor_tensor_scan(out=bd[:, T_ - 1::-1], data0=djb,
                                         data1=ek[:, T_ - 1::-1],
                                         initial=0.0, op0=ALU.mult,
                                         op1=ALU.add)
            return ek, ekv, l1p, diagc, fd, bd

        pending = load_slabs(0)
        for j in range(NCT):
            cj = c1[:, j:j + 1]
            djb = decay[:, j:j + 1].broadcast_to((P, T_))
            cur_slabs = pending
            if j + 1 < NCT:
                pending = load_slabs(j + 1)
            ek, ekv, l1p, diagc, fd, bd = run_front(j, cur_slabs)

            # ---- den on Tensor: pd = diag(c1)@ek + I@Fd + I@Bd (PSUM),
            #      lnden = Ln(pd) chunkwise on Scalar ----
            lnden = mid.tile([P, T_], f32, tag="lnden")
            for q in range(NCH):
                sl = slice(q * CH, (q + 1) * CH)
                pd = psdn.tile([P, CH], f32, tag="pd")
                nc.tensor.matmul(pd[:], diagc[:], ek[:, sl],
                                 start=True, stop=False)
                nc.tensor.matmul(pd[:], ident[:], fd[:, sl],
                                 start=False, stop=False)
                nc.tensor.matmul(pd[:], ident[:], bd[:, sl],
                                 start=False, stop=True)
                nc.scalar.activation(out=lnden[:, sl], in_=pd[:], func=AF.Ln)

            # ---- num scans + assembly on Vector ----
            fn = scano.tile([P, T_], f32, tag="fn")
            bn = scano.tile([P, T_], f32, tag="bn")
            nc.vector.tensor_tensor_scan(out=fn[:], data0=djb, data1=ekv[:],
                                         initial=0.0, op0=ALU.mult,
                                         op1=ALU.add)
            nc.vector.tensor_tensor_scan(out=bn[:, T_ - 1::-1], data0=djb,
                                         data1=ekv[:, T_ - 1::-1],
                                         initial=0.0, op0=ALU.mult,
                                         op1=ALU.add)
            # a1 = c1*ekv + Fn (into fn); num = a1 + Bn (into bn)
            nc.vector.scalar_tensor_tensor(out=fn[:], in0=ekv[:], scalar=cj,
                                           in1=fn[:], op0=ALU.mult,
                                           op1=ALU.add)
            nc.vector.tensor_tensor(bn[:], fn[:], bn[:], ALU.add)

            # ---- s = lnden + l1p (V, into l1p) ----
            nc.vector.tensor_tensor(l1p[:], lnden[:], l1p[:], ALU.add)
            # ---- expo = exp(-s + lnscale) (Scalar, into lnden) ----
            nc.scalar.activation(out=lnden[:], in_=l1p[:], func=AF.Exp,
                                 scale=-1.0, bias=lnscale[:, 0:1])
            # ---- o1 = num * expo (V, into fn) ----
            nc.vector.tensor_tensor(fn[:], bn[:], lnden[:], ALU.mult)

            # ---- transpose out + copy + DMA ----
            oslab = oslabs.tile([P, NT, P], f32, tag="oslab")
            for q in range(NCH):
                po = pso.tile([P, CH], f32, tag="po")
                for s in range(BPC):
                    bs = slice(s * P, (s + 1) * P)
                    tcb = q * BPC + s
                    nc.tensor.transpose(po[:, bs],
                                        fn[:, tcb * P:(tcb + 1) * P],
                                        ident[:])
                nc.scalar.activation(out=oslab[:, q * BPC:(q + 1) * BPC, :],
                                     in_=po[:], func=AF.Copy)
            nc.sync.dma_start(out=slab_src(out, j), in_=oslab[:])


def build_module(T_=T, C_=C, TD_=TD):
    nc = bacc.Bacc("TRN2", target_bir_lowering=False, debug=False)
    r = nc.dram_tensor("r", [T_, C_], f32, kind="ExternalInput").ap()
    k = nc.dram_tensor("k", [T_, C_], f32, kind="ExternalInput").ap()
    v = nc.dram_tensor("v", [T_, C_], f32, kind="ExternalInput").ap()
    w = nc.dram_tensor("w", [C_], f32, kind="ExternalInput").ap()
    u = nc.dram_tensor("u", [C_], f32, kind="ExternalInput").ap()
    te = nc.dram_tensor("time_emb", [TD_], f32, kind="ExternalInput").ap()
    out = nc.dram_tensor("out", [T_, C_], f32, kind="ExternalOutput").ap()
    with tile.TileContext(nc) as tc:
        _body(tc, out, r, k, v, w, u, te, T_, C_, TD_)
    nc.compile()
    return nc


_nc_cache = None


def run_full(r, k, v, w, u, time_emb, trace=False, **spmd_kwargs):
    """Run on 8 cores; returns (output [B,T,C], BassKernelResults)."""
    global _nc_cache
    if _nc_cache is None:
        _nc_cache = build_module()
    nc = _nc_cache
    r = np.asarray(r, dtype=np.float32)
    k = np.asarray(k, dtype=np.float32)
    v = np.asarray(v, dtype=np.float32)
    w = np.asarray(w, dtype=np.float32)
    u = np.asarray(u, dtype=np.float32)
    time_emb = np.asarray(time_emb, dtype=np.float32)
    in_maps = [
        {
            "r": np.ascontiguousarray(r[b]),
            "k": np.ascontiguousarray(k[b]),
            "v": np.ascontiguousarray(v[b]),
            "w": np.ascontiguousarray(w),
            "u": np.ascontiguousarray(u),
            "time_emb": np.ascontiguousarray(time_emb[b]),
        }
        for b in range(B)
    ]
    res = run_bass_kernel_spmd(nc, in_maps, core_ids=list(range(N_CORES)),
                               trace=trace, **spmd_kwargs)
    out = np.stack([res.results[b]["out"] for b in range(B)], axis=0)
    return out, res


def kernel(r, k, v, w, u, time_emb, **extra):
    out, _ = run_full(r, k, v, w, u, time_emb)
    return out


# revision 12
# speedup vs baseline: 1.1818x; 1.1818x over previous
"""Bi-directional WKV (RWKV-style) kernel for Trainium2, 8-core batch-parallel.

Math per (b, t, c):
    tf_b     = sigmoid(sum(time_emb[b]))
    decay_bc = exp(-exp(w_c)) * (0.5 + 0.5*tf_b)
    ek   = exp(k);  ekv = ek * v
    Fn_t = decay*Fn_{t-1} + ekv_t    (forward inclusive scan over T)
    Fd_t = decay*Fd_{t-1} + ek_t
    Bn_t = decay*Bn_{t+1} + ekv_t    (backward inclusive scan)
    Bd_t = decay*Bd_{t+1} + ek_t
    num  = Fn + Bn + (e^u - 1)*ekv
    den  = Fd + Bd + (e^u - 1)*ek
    out  = sigmoid(r) * num/den * (0.8 + 0.2*tf_b)

Engine split per 128-channel ctile. GpSimd is deliberately UNUSED in the
steady state: concurrent GpSimd elementwise traffic slows Vector scans by
~1.8x (mutual SBUF contention, measured), so offloading to it is a net
loss. Split:
  Vector : ekv mult, the 4 scans, num stt+add, s add, o1 mult.
  Tensor : 128x128 input transposes (k,v,r), output transposes, and the
           den assembly as PSUM-accumulated matmuls
           (diag(c1)@ek + I@Fd + I@Bd).
  Scalar : exp(k), exp(-r), ln(1+e^-r), ln(den) from PSUM,
           expo = exp(-s + lnscale), PSUM->SBUF out copy.
"""

import numpy as np
from contextlib import ExitStack

import concourse.bass as bass
import concourse.bacc as bacc
import concourse.tile as tile
from concourse import mybir
from concourse.bass_utils import run_bass_kernel_spmd
from concourse.masks import make_identity

from concourse.hw_specs import get_activation_tables


def _pin_act_tables():
    tabs = get_activation_tables("gen3")
    keep = {"natural_log_exp_and_others", "sigmoid_and_friends"}
    for name in list(tabs):
        if name not in keep:
            tabs[name] = set()


_pin_act_tables()

B, T, C, TD = 8, 2048, 2048, 512
P = 128
N_CORES = 8
f32 = mybir.dt.float32
ALU = mybir.AluOpType
AF = mybir.ActivationFunctionType


def _body(tc, out, r, k, v, w, u, te, T_, C_, TD_):
    nc = tc.nc
    NT = T_ // P   # number of t-chunks in a slab
    NCT = C_ // P  # number of c-tiles

    with ExitStack() as ctx:
        consts = ctx.enter_context(tc.tile_pool(name="consts", bufs=1))
        slabs = ctx.enter_context(tc.tile_pool(name="slabs", bufs=2))
        oslabs = ctx.enter_context(tc.tile_pool(name="oslabs", bufs=2))
        front = ctx.enter_context(tc.tile_pool(name="front", bufs=2))
        scano = ctx.enter_context(tc.tile_pool(name="scano", bufs=2))
        mid = ctx.enter_context(tc.tile_pool(name="mid", bufs=1))
        stage = ctx.enter_context(tc.tile_pool(name="stage", bufs=2))
        psin = ctx.enter_context(tc.tile_pool(name="psin", bufs=2,
                                              space="PSUM"))
        psdn = ctx.enter_context(tc.tile_pool(name="psdn", bufs=1,
                                              space="PSUM"))
        pso = ctx.enter_context(tc.tile_pool(name="pso", bufs=1,
                                             space="PSUM"))

        ident = consts.tile([P, P], f32)
        make_identity(nc, ident[:])

        # ---- per-batch time factor, on all 128 partitions ----
        te_t = consts.tile([P, TD_], f32)
        te_b = bass.AP(tensor=te.tensor, offset=te.offset,
                       ap=[[0, P]] + list(te.ap))
        nc.gpsimd.dma_start(out=te_t[:], in_=te_b)
        ssum = consts.tile([P, 1], f32)
        nc.vector.tensor_reduce(out=ssum[:], in_=te_t[:],
                                axis=mybir.AxisListType.X, op=ALU.add)
        tf = consts.tile([P, 1], f32)
        nc.scalar.activation(out=tf[:], in_=ssum[:], func=AF.Sigmoid)
        scale_b = consts.tile([P, 1], f32)   # 0.8 + 0.2*tf
        nc.vector.tensor_scalar(out=scale_b[:], in0=tf[:], scalar1=0.2,
                                scalar2=0.8, op0=ALU.mult, op1=ALU.add)
        htf = consts.tile([P, 1], f32)       # 0.5 + 0.5*tf
        nc.vector.tensor_scalar(out=htf[:], in0=tf[:], scalar1=0.5,
                                scalar2=0.5, op0=ALU.mult, op1=ALU.add)

        # ---- per-channel constants, [128, NCT] ----
        wt = consts.tile([P, NCT], f32)
        nc.gpsimd.dma_start(out=wt[:], in_=w.rearrange("(j p) -> p j", p=P))
        ut = consts.tile([P, NCT], f32)
        nc.gpsimd.dma_start(out=ut[:], in_=u.rearrange("(j p) -> p j", p=P))
        ew = consts.tile([P, NCT], f32)
        nc.scalar.activation(out=ew[:], in_=wt[:], func=AF.Exp)        # e^w
        dec0 = consts.tile([P, NCT], f32)
        nc.scalar.activation(out=dec0[:], in_=ew[:], func=AF.Exp,
                             scale=-1.0)                               # e^-e^w
        decay = consts.tile([P, NCT], f32)
        nc.vector.tensor_scalar(out=decay[:], in0=dec0[:],
                                scalar1=htf[:, 0:1], scalar2=None,
                                op0=ALU.mult)
        eu = consts.tile([P, NCT], f32)
        nc.scalar.activation(out=eu[:], in_=ut[:], func=AF.Exp)
        c1 = consts.tile([P, NCT], f32)      # e^u - 1
        nc.vector.tensor_scalar(out=c1[:], in0=eu[:], scalar1=1.0,
                                scalar2=None, op0=ALU.subtract)
        lnscale = consts.tile([P, 1], f32)   # ln(0.8 + 0.2*tf)
        nc.scalar.activation(out=lnscale[:], in_=scale_b[:], func=AF.Ln)

        # DRAM views: (tc tp) (j cc) -> tp tc j cc
        def slab_src(ap, j):
            return ap.rearrange("(tc tp) (j cc) -> tp tc j cc",
                                tp=P, cc=P)[:, :, j, :]

        CH = 512          # psum chunk width (one bank)
        NCH = T_ // CH    # chunks per ctile
        BPC = CH // P     # 128-blocks per chunk

        def load_slabs(j):
            kslab = slabs.tile([P, NT, P], f32, tag="kslab")
            vslab = slabs.tile([P, NT, P], f32, tag="vslab")
            rslab = slabs.tile([P, NT, P], f32, tag="rslab")
            nc.sync.dma_start(out=kslab[:], in_=slab_src(k, j))
            nc.sync.dma_start(out=vslab[:], in_=slab_src(v, j))
            nc.sync.dma_start(out=rslab[:], in_=slab_src(r, j))
            return kslab, vslab, rslab

        def run_front(j, slabs3):
            """Transposes + ek/ekv/l1p + the den scans (fd, bd)."""
            kslab, vslab, rslab = slabs3
            cj = c1[:, j:j + 1]
            djb = decay[:, j:j + 1].broadcast_to((P, T_))
            ek = front.tile([P, T_], f32, tag="ek")
            ekv = front.tile([P, T_], f32, tag="ekv")
            l1p = front.tile([P, T_], f32, tag="l1p")
            diagc = front.tile([P, P], f32, tag="diagc")
            nc.vector.tensor_scalar(out=diagc[:], in0=ident[:], scalar1=cj,
                                    scalar2=None, op0=ALU.mult)
            for q in range(NCH):
                sl = slice(q * CH, (q + 1) * CH)
                pk = psin.tile([P, CH], f32, tag="pk")
                pv = psin.tile([P, CH], f32, tag="pv")
                pr = psin.tile([P, CH], f32, tag="pr")
                for s in range(BPC):
                    tcb = q * BPC + s
                    bs = slice(s * P, (s + 1) * P)
                    nc.tensor.transpose(pk[:, bs], kslab[:, tcb, :], ident[:])
                    nc.tensor.transpose(pv[:, bs], vslab[:, tcb, :], ident[:])
                    nc.tensor.transpose(pr[:, bs], rslab[:, tcb, :], ident[:])
                nc.scalar.activation(out=ek[:, sl], in_=pk[:], func=AF.Exp)
                # stage v to SBUF on Scalar so the Vector mult is all-SBUF
                # (PSUM-read tt measured ~1.4x slower than SBUF-read)
                vS = stage.tile([P, CH], f32, tag="vS")
                nc.scalar.activation(out=vS[:], in_=pv[:], func=AF.Copy)
                nc.vector.tensor_tensor(ekv[:, sl], ek[:, sl], vS[:],
                                        ALU.mult)
                nc.scalar.activation(out=l1p[:, sl], in_=pr[:], func=AF.Exp,
                                     scale=-1.0)
                nc.scalar.activation(out=l1p[:, sl], in_=l1p[:, sl],
                                     func=AF.Ln, bias=1.0)
            fd = scano.tile([P, T_], f32, tag="fd")
            bd = scano.tile([P, T_], f32, tag="bd")
            nc.vector.tensor_tensor_scan(out=fd[:], data0=djb, data1=ek[:],
                                         initial=0.0, op0=ALU.mult,
                                         op1=ALU.add)
            nc.vector.tensor_tensor_scan(out=bd[:, T_ - 1::-1], data0=djb,
                                         data1=ek[:, T_ - 1::-1],
                                         initial=0.0, op0=ALU.mult,
                                         op1=ALU.add)
            return ek, ekv, l1p, diagc, fd, bd

        def epilogue(o1t, j):
            # transpose out + copy + DMA for ctile j (o1 lives in o1t)
            oslab = oslabs.tile([P, NT, P], f32, tag="oslab")
            for q in range(NCH):
                po = pso.tile([P, CH], f32, tag="po")
                for s in range(BPC):
                    bs = slice(s * P, (s + 1) * P)
                    tcb = q * BPC + s
                    nc.tensor.transpose(po[:, bs],
                                        o1t[:, tcb * P:(tcb + 1) * P],
                                        ident[:])
                nc.scalar.activation(out=oslab[:, q * BPC:(q + 1) * BPC, :],
                                     in_=po[:], func=AF.Copy)
            nc.sync.dma_start(out=slab_src(out, j), in_=oslab[:])

        pending = load_slabs(0)
        prev_o1 = None
        for j in range(NCT):
            cj = c1[:, j:j + 1]
            djb = decay[:, j:j + 1].broadcast_to((P, T_))
            cur_slabs = pending
            if j + 1 < NCT:
                pending = load_slabs(j + 1)
            ek, ekv, l1p, diagc, fd, bd = run_front(j, cur_slabs)
            # ctile j-1's output epilogue is issued AFTER ctile j's input
            # transposes so the in-order Tensor engine never makes the next
            # ctile's front wait behind output transposes.
            if prev_o1 is not None:
                epilogue(prev_o1, j - 1)

            # ---- den on Tensor: pd = diag(c1)@ek + I@Fd + I@Bd (PSUM),
            #      lnden = Ln(pd) chunkwise on Scalar ----
            lnden = mid.tile([P, T_], f32, tag="lnden")
            for q in range(NCH):
                sl = slice(q * CH, (q + 1) * CH)
                pd = psdn.tile([P, CH], f32, tag="pd")
                nc.tensor.matmul(pd[:], diagc[:], ek[:, sl],
                                 start=True, stop=False)
                nc.tensor.matmul(pd[:], ident[:], fd[:, sl],
                                 start=False, stop=False)
                nc.tensor.matmul(pd[:], ident[:], bd[:, sl],
                                 start=False, stop=True)
                nc.scalar.activation(out=lnden[:, sl], in_=pd[:], func=AF.Ln)

            # ---- num scans + assembly on Vector ----
            fn = scano.tile([P, T_], f32, tag="fn")
            bn = scano.tile([P, T_], f32, tag="bn")
            nc.vector.tensor_tensor_scan(out=fn[:], data0=djb, data1=ekv[:],
                                         initial=0.0, op0=ALU.mult,
                                         op1=ALU.add)
            nc.vector.tensor_tensor_scan(out=bn[:, T_ - 1::-1], data0=djb,
                                         data1=ekv[:, T_ - 1::-1],
                                         initial=0.0, op0=ALU.mult,
                                         op1=ALU.add)
            # a1 = c1*ekv + Fn (into fn); num = a1 + Bn (into bn)
            nc.vector.scalar_tensor_tensor(out=fn[:], in0=ekv[:], scalar=cj,
                                           in1=fn[:], op0=ALU.mult,
                                           op1=ALU.add)
            nc.vector.tensor_tensor(bn[:], fn[:], bn[:], ALU.add)

            # ---- s = lnden + l1p (V, into l1p) ----
            nc.vector.tensor_tensor(l1p[:], lnden[:], l1p[:], ALU.add)
            # ---- expo = exp(-s + lnscale) (Scalar, into lnden) ----
            nc.scalar.activation(out=lnden[:], in_=l1p[:], func=AF.Exp,
                                 scale=-1.0, bias=lnscale[:, 0:1])
            # ---- o1 = num * expo (V, into fn) ----
            nc.vector.tensor_tensor(fn[:], bn[:], lnden[:], ALU.mult)
            prev_o1 = fn

        epilogue(prev_o1, NCT - 1)


def build_module(T_=T, C_=C, TD_=TD):
    nc = bacc.Bacc("TRN2", target_bir_lowering=False, debug=False)
    r = nc.dram_tensor("r", [T_, C_], f32, kind="ExternalInput").ap()
    k = nc.dram_tensor("k", [T_, C_], f32, kind="ExternalInput").ap()
    v = nc.dram_tensor("v", [T_, C_], f32, kind="ExternalInput").ap()
    w = nc.dram_tensor("w", [C_], f32, kind="ExternalInput").ap()
    u = nc.dram_tensor("u", [C_], f32, kind="ExternalInput").ap()
    te = nc.dram_tensor("time_emb", [TD_], f32, kind="ExternalInput").ap()
    out = nc.dram_tensor("out", [T_, C_], f32, kind="ExternalOutput").ap()
    with tile.TileContext(nc) as tc:
        _body(tc, out, r, k, v, w, u, te, T_, C_, TD_)
    nc.compile()
    return nc


_nc_cache = None


def run_full(r, k, v, w, u, time_emb, trace=False, **spmd_kwargs):
    """Run on 8 cores; returns (output [B,T,C], BassKernelResults)."""
    global _nc_cache
    if _nc_cache is None:
        _nc_cache = build_module()
    nc = _nc_cache
    r = np.asarray(r, dtype=np.float32)
    k = np.asarray(k, dtype=np.float32)
    v = np.asarray(v, dtype=np.float32)
    w = np.asarray(w, dtype=np.float32)
    u = np.asarray(u, dtype=np.float32)
    time_emb = np.asarray(time_emb, dtype=np.float32)
    in_maps = [
        {
            "r": np.ascontiguousarray(r[b]),
            "k": np.ascontiguousarray(k[b]),
            "v": np.ascontiguousarray(v[b]),
            "w": np.ascontiguousarray(w),
            "u": np.ascontiguousarray(u),
            "time_emb": np.ascontiguousarray(time_emb[b]),
        }
        for b in range(B)
    ]
    res = run_bass_kernel_spmd(nc, in_maps, core_ids=list(range(N_CORES)),
                               trace=trace, **spmd_kwargs)
    out = np.stack([res.results[b]["out"] for b in range(B)], axis=0)
    return out, res


def kernel(r, k, v, w, u, time_emb, **extra):
    out, _ = run_full(r, k, v, w, u, time_emb)
    return out
